# revision 2
# baseline (speedup 1.0000x reference)
"""Trainium2 Bass kernel for modulated deformable conv2d (torchvision semantics).

Problem (hardcoded): input [4,64,128,128] f32, offset [4,18,128,128], mask
[4,9,128,128], weight [64,64,3,3], bias [64]; stride 1, pad 1, dil 1.

Strategy (8 NeuronCores, SPMD, no collectives):
  - Shard: core = (sample b, row-half h).  Each core computes output rows
    [h*64, h*64+64) of sample b => out slice [64, 8192] f32.
  - Bilinear gather via difference planes:
        val = I[y0,x0] + lx*D01[y0,x0] + ly*D10[y0,x0] + lx*ly*D11[y0,x0]
    Each sample point is a SINGLE 512-byte-row gather from an SBUF-resident
    table, fetched with GPSIMD dma_gather (transpose mode) so data lands
    channels-on-partitions.  Table row layout (256 bf16):
        [I(c0:32) D01(c0:32) D10(c0:32) D11(c0:32) | same for c32:64]
    so transposed rank r, partition p = (chan-half r, plane p//32, ch p%32).
  - Gather indices and beta planes (m, m*lx, m*ly, m*lx*ly) are computed on
    host and shipped as inputs; device replicates the 16-wrap indices.
  - Per 512-pixel block x 9 taps: one contraction-4 PE matmul broadcasts all
    4 betas to their 32-partition groups (PSUM), DVE multiplies the gathered
    rows by betas (reading PSUM directly), and the conv contraction
    (plane-sum + channels -> O) runs as an accumulated PE matmul chain.
  - Gathers use prepare_only + trigger so GPSIMD desc-gen for block t+1
    overlaps DMA execution of block t and compute of block t-1.
"""

import sys

if "/opt/trn_rl_repo" not in sys.path:
    sys.path.insert(0, "/opt/trn_rl_repo")

import numpy as np
import ml_dtypes

BF16 = ml_dtypes.bfloat16

# problem dims
B, C, H, W = 4, 64, 128, 128
O, K = 64, 9
PAD = 8                     # gather window margin (|offset| <= ~6.8 required)
TG = H + 2 * PAD + 1        # 145: table grid covers y,x in [-PAD, H+PAD]
GEXT = TG + 1               # 146: extended image grid (D planes read +1)
NROWS = TG * TG             # 21025
RANKS = (NROWS + 127) // 128  # 165
NROWS_PAD = RANKS * 128     # 21120
NPIX = H * W // 2           # 8192 output pixels per core
NBLK = 16                   # pixel blocks per core
BLK = NPIX // NBLK          # 512 pixels per block
CALL = K * BLK              # 4608 gather indices per block (all 9 taps)
NSAMP = K * NPIX            # 73728 sample points per core
MAGIC = 12582912.0          # kept for import compat

L1F = NSAMP // 128          # 576
L2F = NSAMP // 16           # 4608

_CACHE = {}

USE_PREP = False            # prep+trigger measured as pure overhead (+1.7us/call)


def _split_excess_waits(nc, limit=1):
    """Walrus in this image caps sync-wait commands per instruction; hoist
    excess waits onto preceding same-engine NoOps (engine streams are
    in-order, so blocking earlier on a prefix of the waits is equivalent)."""
    from concourse import mybir

    n = 0
    for fn in nc.m.functions:
        for blk in fn.blocks:
            new = []
            for inst in blk.instructions:
                si = inst.sync_info
                if si is not None and len(si.on_wait) > limit:
                    waits = list(si.on_wait)
                    head, keep = waits[:-limit], waits[-limit:]
                    for i in range(0, len(head), limit):
                        n += 1
                        new.append(mybir.InstNoOp(
                            name=f"waitsplit_{n}",
                            sync_info=mybir.SyncInfo(
                                on_wait=head[i:i + limit], on_update=[]),
                            bass_nofuse=True,
                            engine=inst.engine,
                        ))
                    inst.sync_info = mybir.SyncInfo(
                        on_wait=keep, on_update=list(si.on_update))
                new.append(inst)
            blk.instructions = new


def _build_program():
    import concourse.bass as bass
    import concourse.tile as tile
    from concourse import mybir

    f32 = mybir.dt.float32
    bf16 = mybir.dt.bfloat16
    i16 = mybir.dt.int16

    nc = bass.Bass("TRN2", target_bir_lowering=False, debug=False,
                   enable_asserts=False, dynamic_dma_scratch_size=32768)

    tab_d = nc.dram_tensor("tab", [128, RANKS, 4 * C], bf16, kind="ExternalInput")
    idx_d = nc.dram_tensor("idx", [16, L2F], i16, kind="ExternalInput")
    bw_d = nc.dram_tensor("bw", [128, 4, L1F], bf16, kind="ExternalInput")
    wt_d = nc.dram_tensor("wt", [128, 2, K * O], bf16, kind="ExternalInput")
    ones4_d = nc.dram_tensor("ones4", [4, 128], bf16, kind="ExternalInput")
    bias_d = nc.dram_tensor("bias", [O, 1], f32, kind="ExternalInput")
    out_d = nc.dram_tensor("out", [O, NPIX], f32, kind="ExternalOutput")

    from concourse import library_config

    with tile.TileContext(nc) as tc:
        nc.gpsimd.load_library(library_config.mlp)   # provides DMAGatherAnt
        with tc.tile_pool(name="const", bufs=1) as cp:
            tab = cp.tile([128, RANKS, 4 * C], bf16, tag="tab")
            nc.sync.dma_start(tab[:], tab_d.ap())

            w_sb = cp.tile([128, 2, K * O], bf16, tag="wsb")
            nc.sync.dma_start(w_sb[:], wt_d.ap())

            bias_sb = cp.tile([O, 1], f32, tag="bias")
            nc.sync.dma_start(bias_sb[:], bias_d.ap())

            ones4 = cp.tile([4, 128], bf16, tag="ones4")
            nc.sync.dma_start(ones4[:], ones4_d.ap())

            bw = cp.tile([128, 4, L1F], bf16, tag="bw")
            nc.sync.dma_start(bw[:], bw_d.ap())

            idxr = cp.tile([128, L2F], i16, tag="idxr")
            nc.sync.dma_start(idxr[0:16, :], idx_d.ap())
            for g in range(1, 8):
                nc.sync.dma_start(idxr[16 * g:16 * (g + 1), :], idxr[0:16, :])

            r4608 = nc.gpsimd.to_reg(CALL)
            dma_sem = nc.alloc_semaphore("gat_dma") if USE_PREP else None

            with (
                tc.tile_pool(name="g", bufs=3) as gp,
                tc.tile_pool(name="bst", bufs=2) as bstp,
                tc.tile_pool(name="bpsum", bufs=4, space="PSUM") as bpp,
                tc.tile_pool(name="opsum", bufs=2, space="PSUM") as opp,
                tc.tile_pool(name="val", bufs=4) as vp,
                tc.tile_pool(name="ob", bufs=2) as obp,
            ):
                for t in range(NBLK):
                    g = gp.tile([128, 2, CALL], bf16, tag="g")
                    s0 = t * (CALL // 16)
                    if USE_PREP:
                        nc.gpsimd.dma_gather(
                            g[:], tab[:], idxr[:, s0:s0 + CALL // 16],
                            CALL, r4608, 4 * C,
                            transpose=True, single_packet=False,
                            sbuf_tokens_per_rank=128,
                            sbuf_free_dim_per_rank=512,
                            prepare_only=True, sem=dma_sem)
                        nc.gpsimd.trigger_dma(count=None)
                    else:
                        nc.gpsimd.dma_gather(
                            g[:], tab[:], idxr[:, s0:s0 + CALL // 16],
                            CALL, r4608, 4 * C,
                            transpose=True, single_packet=False,
                            sbuf_tokens_per_rank=128,
                            sbuf_free_dim_per_rank=512)

                    # stage this block's beta rows at partitions 0-3 for PE rhs
                    bst = bstp.tile([4, CALL], bf16, tag="bst")
                    src = bw[8 * t:8 * (t + 1), :, :]
                    for q in range(4):
                        nc.sync.dma_start(bst[q:q + 1, :], src[:, q, :])

                    ops = opp.tile([O, BLK], f32, tag="ops")
                    for k in range(K):
                        sl = slice(k * BLK, (k + 1) * BLK)
                        bp = bpp.tile([128, BLK], f32, tag="bp")
                        nc.tensor.matmul(bp[:], ones4[:], bst[:, sl],
                                         start=True, stop=True)
                        vA = vp.tile([128, BLK], bf16, tag="v")
                        nc.vector.tensor_mul(vA[:], g[:, 0, sl], bp[:])
                        vB = vp.tile([128, BLK], bf16, tag="v")
                        nc.vector.tensor_mul(vB[:], g[:, 1, sl], bp[:])
                        nc.tensor.matmul(ops[:], w_sb[:, 0, k * O:(k + 1) * O],
                                         vA[:], start=(k == 0), stop=False)
                        nc.tensor.matmul(ops[:], w_sb[:, 1, k * O:(k + 1) * O],
                                         vB[:], start=False, stop=(k == K - 1))

                    ob = obp.tile([O, BLK], f32, tag="ob")
                    nc.scalar.add(ob[:], ops[:], bias_sb[:, 0:1])
                    nc.sync.dma_start(out_d.ap()[:, t * BLK:(t + 1) * BLK], ob[:])

    _split_excess_waits(nc)
    from concourse.library_overlay import lower_extended_insts
    lower_extended_insts(nc)
    return nc


def _host_prep(input, offset, mask, weight, bias):
    x = np.asarray(input, np.float32)
    off = np.asarray(offset, np.float32)
    msk = np.asarray(mask, np.float32)
    w = np.asarray(weight, np.float32)
    b = np.asarray(bias, np.float32)

    amax = float(np.abs(off).max())
    if amax >= PAD - 1.2:
        raise ValueError(f"offset magnitude {amax} exceeds supported window")

    f32 = np.float32

    # per-sample gather tables; row = [half0: I,D01,D10,D11 x 32ch | half1]
    tabs = []
    for bb in range(B):
        E = np.zeros((C, GEXT, GEXT), f32)
        E[:, PAD:PAD + H, PAD:PAD + W] = x[bb]
        Eb = E.astype(BF16).astype(f32)
        D01 = np.zeros((C, GEXT, GEXT), f32)
        D01[:, :, :-1] = Eb[:, :, 1:] - Eb[:, :, :-1]
        D10 = np.zeros((C, GEXT, GEXT), f32)
        D10[:, :-1, :] = Eb[:, 1:, :] - Eb[:, :-1, :]
        D11 = np.zeros((C, GEXT, GEXT), f32)
        D11[:, :-1, :-1] = (Eb[:, 1:, 1:] - Eb[:, 1:, :-1]
                            - Eb[:, :-1, 1:] + Eb[:, :-1, :-1])
        planes = np.stack([Eb, D01, D10, D11], 0)[:, :, :TG, :TG]  # [4,C,TG,TG]
        # -> [TG, TG, half, plane, 32] -> rows [NROWS, 256]
        arr = planes.reshape(4, 2, 32, TG, TG).transpose(3, 4, 1, 0, 2)
        rows = np.zeros((NROWS_PAD, 4 * C), BF16)
        rows[:NROWS] = arr.reshape(NROWS, 4 * C).astype(BF16)
        tabdram = np.ascontiguousarray(
            rows.reshape(RANKS, 128, 4 * C).transpose(1, 0, 2))   # [128,165,256]
        tabs.append(tabdram)

    # conv lhsT per rank-half: [128 part=(plane,ch%32), K, O]
    wk = w.reshape(O, C, K)                      # [O, C, K]
    wtA = np.zeros((128, K, O), f32)
    wtB = np.zeros((128, K, O), f32)
    for p in range(128):
        c = p % 32
        wtA[p] = wk[:, c, :].T                   # [K, O]
        wtB[p] = wk[:, 32 + c, :].T
    wt_both = np.stack(
        [wtA.reshape(128, K * O), wtB.reshape(128, K * O)], axis=1)  # [128,2,KO]

    ones4c = np.zeros((4, 128), f32)
    for q in range(4):
        ones4c[q, 32 * q:32 * (q + 1)] = 1.0
    bias2 = np.ascontiguousarray(b.reshape(O, 1))

    karr = np.arange(K)
    p = np.arange(NPIX)
    ylo = p // W
    xloc = p % W

    def tojd(a):  # [K, NPIX] -> flat j order (t, k, ptilde)
        return np.ascontiguousarray(
            a.reshape(K, NBLK, BLK).transpose(1, 0, 2).reshape(-1))

    in_maps = []
    for core in range(8):
        bb, h = divmod(core, 2)
        yg = h * 64 + ylo                                   # [NPIX] global y
        offv = off[bb].reshape(K, 2, H, W)
        oy_kp = offv[:, 0][:, yg, xloc]                     # [K, NPIX]
        ox_kp = offv[:, 1][:, yg, xloc]
        m_kp = msk[bb][:, yg, xloc]
        by = yg[None, :] - 1 + (karr // 3)[:, None]
        bx = xloc[None, :] - 1 + (karr % 3)[:, None]

        y0 = np.floor(oy_kp)
        x0 = np.floor(ox_kp)
        ly = oy_kp - y0
        lx = ox_kp - x0
        idx_kp = (y0 + by + PAD) * TG + (x0 + bx + PAD)     # [K, NPIX]

        idxj = tojd(idx_kp).astype(np.int64)
        assert idxj.min() >= 0 and idxj.max() < NROWS
        idx16 = np.ascontiguousarray(
            idxj.reshape(NSAMP // 16, 16).T).astype(np.int16)

        mj = tojd(m_kp).astype(f32)
        lxj = tojd(lx).astype(f32)
        lyj = tojd(ly).astype(f32)
        # bw[q, plane, e]: j = (q//8)*4608 + (q%8)*576 + e
        bws = np.stack([mj, mj * lxj, mj * lyj, mj * lxj * lyj], 0)  # [4, NSAMP]
        bw_host = np.ascontiguousarray(
            bws.reshape(4, 128, L1F).transpose(1, 0, 2)).astype(BF16)

        in_maps.append({
            "tab": tabs[bb],
            "idx": idx16,
            "bw": bw_host,
            "wt": wt_both.astype(BF16),
            "ones4": ones4c.astype(BF16),
            "bias": bias2,
        })
    return in_maps


def _install_ntff_shim():
    """Provide antenv.axon_hooks (missing in this image) so trace=True works."""
    import types
    if "antenv.axon_hooks" in sys.modules:
        return
    sys.path.insert(0, "/root/.axon_site")
    from trn_agent_boot.trn_boot import _ntff_profile_via_ctypes
    hook = _ntff_profile_via_ctypes("/opt/axon/libaxon_pjrt.so")
    mod = types.ModuleType("antenv.axon_hooks")
    mod.get_axon_ntff_profile_hook = lambda: hook
    mod.set_axon_ntff_profile_hook = lambda h: None
    sys.modules["antenv.axon_hooks"] = mod


def kernel(input, offset, mask, weight, bias, _trace=False):
    if _trace:
        _install_ntff_shim()
    from concourse.bass_utils import run_bass_kernel_spmd

    if "nc" not in _CACHE:
        _CACHE["nc"] = _build_program()
    nc = _CACHE["nc"]

    in_maps = _host_prep(input, offset, mask, weight, bias)
    res = run_bass_kernel_spmd(
        nc, in_maps, core_ids=list(range(8)),
        trace=_trace,
        trace_cores=list(range(8)) if _trace else None,
    )
    kernel.last_results = res

    out = np.empty((B, O, H, W), np.float32)
    for core in range(8):
        bb, h = divmod(core, 2)
        blockout = res.results[core]["out"]       # [64, 8192] f32
        out[bb, :, h * 64:(h + 1) * 64, :] = blockout.reshape(O, 64, W)
    return out


# revision 5
# speedup vs baseline: 1.2993x; 1.2993x over previous
"""Trainium2 Bass kernel for modulated deformable conv2d (torchvision semantics).

Problem (hardcoded): input [4,64,128,128] f32, offset [4,18,128,128], mask
[4,9,128,128], weight [64,64,3,3], bias [64]; stride 1, pad 1, dil 1.

Strategy (8 NeuronCores, SPMD, no collectives):
  - Shard: core = (sample b, row-half h).  Each core computes output rows
    [h*64, h*64+64) of sample b => out slice [64, 8192] f32.
  - Bilinear gather via difference planes:
        val = I[y0,x0] + lx*D01[y0,x0] + ly*D10[y0,x0] + lx*ly*D11[y0,x0]
    Each sample point is a SINGLE 512-byte-row gather from an SBUF-resident
    table, fetched with GPSIMD dma_gather (transpose mode) so data lands
    channels-on-partitions.  Table row layout (256 bf16):
        [I(c0:32) D01(c0:32) D10(c0:32) D11(c0:32) | same for c32:64]
    so transposed rank r, partition p = (chan-half r, plane p//32, ch p%32).
  - Gather indices and beta planes (m, m*lx, m*ly, m*lx*ly) are computed on
    host and shipped as inputs; device replicates the 16-wrap indices.
  - Per 512-pixel block x 9 taps: one contraction-4 PE matmul broadcasts all
    4 betas to their 32-partition groups (PSUM), DVE multiplies the gathered
    rows by betas (reading PSUM directly), and the conv contraction
    (plane-sum + channels -> O) runs as an accumulated PE matmul chain.
  - Gathers use prepare_only + trigger so GPSIMD desc-gen for block t+1
    overlaps DMA execution of block t and compute of block t-1.
"""

import sys

if "/opt/trn_rl_repo" not in sys.path:
    sys.path.insert(0, "/opt/trn_rl_repo")

import numpy as np
import ml_dtypes

BF16 = ml_dtypes.bfloat16

# problem dims
B, C, H, W = 4, 64, 128, 128
O, K = 64, 9
PAD = 8                     # gather window margin (|offset| <= ~6.8 required)
TG = H + 2 * PAD + 1        # 145: table grid covers y,x in [-PAD, H+PAD]
GEXT = TG + 1               # 146: extended image grid (D planes read +1)
NROWS = TG * TG             # 21025
RANKS = (NROWS + 127) // 128  # 165
NROWS_PAD = RANKS * 128     # 21120
NPIX = H * W // 2           # 8192 output pixels per core
NBLK = 16                   # pixel blocks per core
BLK = NPIX // NBLK          # 512 pixels per block
CALL = K * BLK              # 4608 gather indices per block (all 9 taps)
NSAMP = K * NPIX            # 73728 sample points per core
MAGIC = 12582912.0          # kept for import compat

L1F = NSAMP // 128          # 576
L2F = NSAMP // 16           # 4608

_CACHE = {}

USE_PREP = False            # prep+trigger measured as pure overhead (+1.7us/call)


def _split_excess_waits(nc, limit=1):
    """Walrus in this image caps sync-wait commands per instruction; hoist
    excess waits onto preceding same-engine NoOps (engine streams are
    in-order, so blocking earlier on a prefix of the waits is equivalent)."""
    from concourse import mybir

    n = 0
    for fn in nc.m.functions:
        for blk in fn.blocks:
            new = []
            for inst in blk.instructions:
                si = inst.sync_info
                if si is not None and len(si.on_wait) > limit:
                    waits = list(si.on_wait)
                    head, keep = waits[:-limit], waits[-limit:]
                    for i in range(0, len(head), limit):
                        n += 1
                        new.append(mybir.InstNoOp(
                            name=f"waitsplit_{n}",
                            sync_info=mybir.SyncInfo(
                                on_wait=head[i:i + limit], on_update=[]),
                            bass_nofuse=True,
                            engine=inst.engine,
                        ))
                    inst.sync_info = mybir.SyncInfo(
                        on_wait=keep, on_update=list(si.on_update))
                new.append(inst)
            blk.instructions = new


def _build_program():
    import concourse.bass as bass
    import concourse.tile as tile
    from concourse import mybir

    f32 = mybir.dt.float32
    bf16 = mybir.dt.bfloat16
    i16 = mybir.dt.int16

    nc = bass.Bass("TRN2", target_bir_lowering=False, debug=False,
                   enable_asserts=False, dynamic_dma_scratch_size=65536)

    tab_d = nc.dram_tensor("tab", [128, RANKS, 4 * C], bf16, kind="ExternalInput")
    idx_d = nc.dram_tensor("idx", [16, L2F], i16, kind="ExternalInput")
    bw_d = nc.dram_tensor("bw", [128, 4, L1F], bf16, kind="ExternalInput")
    wt_d = nc.dram_tensor("wt", [128, 2, K * O], bf16, kind="ExternalInput")
    ones4_d = nc.dram_tensor("ones4", [4, 128], bf16, kind="ExternalInput")
    bias_d = nc.dram_tensor("bias", [O, 1], f32, kind="ExternalInput")
    out_d = nc.dram_tensor("out", [O, NPIX], f32, kind="ExternalOutput")

    from concourse import library_config

    with tile.TileContext(nc) as tc:
        nc.gpsimd.load_library(library_config.mlp)   # provides DMAGatherAnt
        with tc.tile_pool(name="const", bufs=1) as cp:
            tab = cp.tile([128, RANKS, 4 * C], bf16, tag="tab")
            nc.sync.dma_start(tab[:], tab_d.ap())

            w_sb = cp.tile([128, 2, K * O], bf16, tag="wsb")
            nc.sync.dma_start(w_sb[:], wt_d.ap())

            bias_sb = cp.tile([O, 1], f32, tag="bias")
            nc.sync.dma_start(bias_sb[:], bias_d.ap())

            ones4 = cp.tile([4, 128], bf16, tag="ones4")
            nc.sync.dma_start(ones4[:], ones4_d.ap())

            bw = cp.tile([128, 4, L1F], bf16, tag="bw")
            nc.sync.dma_start(bw[:], bw_d.ap())

            idxr = cp.tile([128, L2F], i16, tag="idxr")
            for grp in range(8):
                nc.sync.dma_start(idxr[16 * grp:16 * (grp + 1), :], idx_d.ap())

            GK = 3                        # taps per gather call
            GCALL = GK * BLK              # 1536 idx/call: 292 ring descs, no stall
            rg = nc.gpsimd.to_reg(GCALL)
            dma_sem = nc.alloc_semaphore("gat_dma") if USE_PREP else None

            with (
                tc.tile_pool(name="g", bufs=3) as gp,
                tc.tile_pool(name="bst", bufs=2) as bstp,
                tc.tile_pool(name="bpsum", bufs=4, space="PSUM") as bpp,
                tc.tile_pool(name="opsum", bufs=2, space="PSUM") as opp,
                tc.tile_pool(name="val", bufs=4) as vp,
                tc.tile_pool(name="ob", bufs=2) as obp,
            ):
                for t in range(NBLK):
                    gs = []
                    for kg in range(K // GK):
                        g = gp.tile([128, 2, GCALL], bf16, tag="g")
                        s0 = t * (CALL // 16) + kg * (GCALL // 16)
                        nc.gpsimd.dma_gather(
                            g[:], tab[:], idxr[:, s0:s0 + GCALL // 16],
                            GCALL, rg, 4 * C,
                            transpose=True, single_packet=False,
                            sbuf_tokens_per_rank=128,
                            sbuf_free_dim_per_rank=512)
                        gs.append(g)

                    # stage this block's beta rows at partitions 0-3 for PE rhs
                    bst = bstp.tile([4, CALL], bf16, tag="bst")
                    src = bw[8 * t:8 * (t + 1), :, :]
                    for q in range(4):
                        nc.sync.dma_start(bst[q:q + 1, :], src[:, q, :])

                    ops = opp.tile([O, BLK], f32, tag="ops")
                    for k in range(K):
                        sl = slice(k * BLK, (k + 1) * BLK)
                        gsl = slice((k % GK) * BLK, (k % GK + 1) * BLK)
                        g = gs[k // GK]
                        bp = bpp.tile([128, BLK], f32, tag="bp")
                        nc.tensor.matmul(bp[:], ones4[:], bst[:, sl],
                                         start=True, stop=True)
                        vA = vp.tile([128, BLK], bf16, tag="v")
                        nc.vector.tensor_mul(vA[:], g[:, 0, gsl], bp[:])
                        vB = vp.tile([128, BLK], bf16, tag="v")
                        nc.vector.tensor_mul(vB[:], g[:, 1, gsl], bp[:])
                        nc.tensor.matmul(ops[:], w_sb[:, 0, k * O:(k + 1) * O],
                                         vA[:], start=(k == 0), stop=False)
                        nc.tensor.matmul(ops[:], w_sb[:, 1, k * O:(k + 1) * O],
                                         vB[:], start=False, stop=(k == K - 1))

                    ob = obp.tile([O, BLK], f32, tag="ob")
                    nc.scalar.add(ob[:], ops[:], bias_sb[:, 0:1])
                    nc.sync.dma_start(out_d.ap()[:, t * BLK:(t + 1) * BLK], ob[:])

    _split_excess_waits(nc)
    from concourse.library_overlay import lower_extended_insts
    lower_extended_insts(nc)
    return nc


def _host_prep(input, offset, mask, weight, bias):
    x = np.asarray(input, np.float32)
    off = np.asarray(offset, np.float32)
    msk = np.asarray(mask, np.float32)
    w = np.asarray(weight, np.float32)
    b = np.asarray(bias, np.float32)

    amax = float(np.abs(off).max())
    if amax >= PAD - 1.2:
        raise ValueError(f"offset magnitude {amax} exceeds supported window")

    f32 = np.float32

    # per-sample gather tables; row = [half0: I,D01,D10,D11 x 32ch | half1]
    tabs = []
    for bb in range(B):
        E = np.zeros((C, GEXT, GEXT), f32)
        E[:, PAD:PAD + H, PAD:PAD + W] = x[bb]
        Eb = E.astype(BF16).astype(f32)
        D01 = np.zeros((C, GEXT, GEXT), f32)
        D01[:, :, :-1] = Eb[:, :, 1:] - Eb[:, :, :-1]
        D10 = np.zeros((C, GEXT, GEXT), f32)
        D10[:, :-1, :] = Eb[:, 1:, :] - Eb[:, :-1, :]
        D11 = np.zeros((C, GEXT, GEXT), f32)
        D11[:, :-1, :-1] = (Eb[:, 1:, 1:] - Eb[:, 1:, :-1]
                            - Eb[:, :-1, 1:] + Eb[:, :-1, :-1])
        planes = np.stack([Eb, D01, D10, D11], 0)[:, :, :TG, :TG]  # [4,C,TG,TG]
        # -> [TG, TG, half, plane, 32] -> rows [NROWS, 256]
        arr = planes.reshape(4, 2, 32, TG, TG).transpose(3, 4, 1, 0, 2)
        rows = np.zeros((NROWS_PAD, 4 * C), BF16)
        rows[:NROWS] = arr.reshape(NROWS, 4 * C).astype(BF16)
        tabdram = np.ascontiguousarray(
            rows.reshape(RANKS, 128, 4 * C).transpose(1, 0, 2))   # [128,165,256]
        tabs.append(tabdram)

    # conv lhsT per rank-half: [128 part=(plane,ch%32), K, O]
    wk = w.reshape(O, C, K)                      # [O, C, K]
    wtA = np.zeros((128, K, O), f32)
    wtB = np.zeros((128, K, O), f32)
    for p in range(128):
        c = p % 32
        wtA[p] = wk[:, c, :].T                   # [K, O]
        wtB[p] = wk[:, 32 + c, :].T
    wt_both = np.stack(
        [wtA.reshape(128, K * O), wtB.reshape(128, K * O)], axis=1)  # [128,2,KO]

    ones4c = np.zeros((4, 128), f32)
    for q in range(4):
        ones4c[q, 32 * q:32 * (q + 1)] = 1.0
    bias2 = np.ascontiguousarray(b.reshape(O, 1))

    karr = np.arange(K)
    p = np.arange(NPIX)
    ylo = p // W
    xloc = p % W

    def tojd(a):  # [K, NPIX] -> flat j order (t, k, ptilde)
        return np.ascontiguousarray(
            a.reshape(K, NBLK, BLK).transpose(1, 0, 2).reshape(-1))

    in_maps = []
    for core in range(8):
        bb, h = divmod(core, 2)
        yg = h * 64 + ylo                                   # [NPIX] global y
        offv = off[bb].reshape(K, 2, H, W)
        oy_kp = offv[:, 0][:, yg, xloc]                     # [K, NPIX]
        ox_kp = offv[:, 1][:, yg, xloc]
        m_kp = msk[bb][:, yg, xloc]
        by = yg[None, :] - 1 + (karr // 3)[:, None]
        bx = xloc[None, :] - 1 + (karr % 3)[:, None]

        y0 = np.floor(oy_kp)
        x0 = np.floor(ox_kp)
        ly = oy_kp - y0
        lx = ox_kp - x0
        idx_kp = (y0 + by + PAD) * TG + (x0 + bx + PAD)     # [K, NPIX]

        idxj = tojd(idx_kp).astype(np.int64)
        assert idxj.min() >= 0 and idxj.max() < NROWS
        idx16 = np.ascontiguousarray(
            idxj.reshape(NSAMP // 16, 16).T).astype(np.int16)

        mj = tojd(m_kp).astype(f32)
        lxj = tojd(lx).astype(f32)
        lyj = tojd(ly).astype(f32)
        # bw[q, plane, e]: j = (q//8)*4608 + (q%8)*576 + e
        bws = np.stack([mj, mj * lxj, mj * lyj, mj * lxj * lyj], 0)  # [4, NSAMP]
        bw_host = np.ascontiguousarray(
            bws.reshape(4, 128, L1F).transpose(1, 0, 2)).astype(BF16)

        in_maps.append({
            "tab": tabs[bb],
            "idx": idx16,
            "bw": bw_host,
            "wt": wt_both.astype(BF16),
            "ones4": ones4c.astype(BF16),
            "bias": bias2,
        })
    return in_maps


def _install_ntff_shim():
    """Provide antenv.axon_hooks (missing in this image) so trace=True works."""
    import types
    if "antenv.axon_hooks" in sys.modules:
        return
    sys.path.insert(0, "/root/.axon_site")
    from trn_agent_boot.trn_boot import _ntff_profile_via_ctypes
    hook = _ntff_profile_via_ctypes("/opt/axon/libaxon_pjrt.so")
    mod = types.ModuleType("antenv.axon_hooks")
    mod.get_axon_ntff_profile_hook = lambda: hook
    mod.set_axon_ntff_profile_hook = lambda h: None
    sys.modules["antenv.axon_hooks"] = mod


def kernel(input, offset, mask, weight, bias, _trace=False):
    if _trace:
        _install_ntff_shim()
    from concourse.bass_utils import run_bass_kernel_spmd

    if "nc" not in _CACHE:
        _CACHE["nc"] = _build_program()
    nc = _CACHE["nc"]

    in_maps = _host_prep(input, offset, mask, weight, bias)
    res = run_bass_kernel_spmd(
        nc, in_maps, core_ids=list(range(8)),
        trace=_trace,
        trace_cores=list(range(8)) if _trace else None,
    )
    kernel.last_results = res

    out = np.empty((B, O, H, W), np.float32)
    for core in range(8):
        bb, h = divmod(core, 2)
        blockout = res.results[core]["out"]       # [64, 8192] f32
        out[bb, :, h * 64:(h + 1) * 64, :] = blockout.reshape(O, 64, W)
    return out


# revision 6
# speedup vs baseline: 1.3093x; 1.0077x over previous
"""Trainium2 Bass kernel for modulated deformable conv2d (torchvision semantics).

Problem (hardcoded): input [4,64,128,128] f32, offset [4,18,128,128], mask
[4,9,128,128], weight [64,64,3,3], bias [64]; stride 1, pad 1, dil 1.

Strategy (8 NeuronCores, SPMD, no collectives):
  - Shard: core = (sample b, row-half h).  Each core computes output rows
    [h*64, h*64+64) of sample b => out slice [64, 8192] f32.
  - Bilinear gather via difference planes:
        val = I[y0,x0] + lx*D01[y0,x0] + ly*D10[y0,x0] + lx*ly*D11[y0,x0]
    Each sample point is a SINGLE 512-byte-row gather from an SBUF-resident
    table, fetched with GPSIMD dma_gather (transpose mode) so data lands
    channels-on-partitions.  Table row layout (256 bf16):
        [I(c0:32) D01(c0:32) D10(c0:32) D11(c0:32) | same for c32:64]
    so transposed rank r, partition p = (chan-half r, plane p//32, ch p%32).
  - Gather indices and beta planes (m, m*lx, m*ly, m*lx*ly) are computed on
    host and shipped as inputs; device replicates the 16-wrap indices.
  - Per 512-pixel block x 9 taps: one contraction-4 PE matmul broadcasts all
    4 betas to their 32-partition groups (PSUM), DVE multiplies the gathered
    rows by betas (reading PSUM directly), and the conv contraction
    (plane-sum + channels -> O) runs as an accumulated PE matmul chain.
  - Gathers use prepare_only + trigger so GPSIMD desc-gen for block t+1
    overlaps DMA execution of block t and compute of block t-1.
"""

import sys

if "/opt/trn_rl_repo" not in sys.path:
    sys.path.insert(0, "/opt/trn_rl_repo")

import numpy as np
import ml_dtypes

BF16 = ml_dtypes.bfloat16

# problem dims
B, C, H, W = 4, 64, 128, 128
O, K = 64, 9
PAD = 8                     # gather window margin (|offset| <= ~6.8 required)
TG = H + 2 * PAD + 1        # 145: table grid covers y,x in [-PAD, H+PAD]
GEXT = TG + 1               # 146: extended image grid (D planes read +1)
NROWS = TG * TG             # 21025
RANKS = (NROWS + 127) // 128  # 165
NROWS_PAD = RANKS * 128     # 21120
NPIX = H * W // 2           # 8192 output pixels per core
NBLK = 16                   # pixel blocks per core
BLK = NPIX // NBLK          # 512 pixels per block
CALL = K * BLK              # 4608 gather indices per block (all 9 taps)
NSAMP = K * NPIX            # 73728 sample points per core
MAGIC = 12582912.0          # kept for import compat

L1F = NSAMP // 128          # 576
L2F = NSAMP // 16           # 4608

_CACHE = {}

USE_PREP = False            # prep+trigger measured as pure overhead (+1.7us/call)


def _split_excess_waits(nc, limit=1):
    """Walrus in this image caps sync-wait commands per instruction; hoist
    excess waits onto preceding same-engine NoOps (engine streams are
    in-order, so blocking earlier on a prefix of the waits is equivalent)."""
    from concourse import mybir

    n = 0
    for fn in nc.m.functions:
        for blk in fn.blocks:
            new = []
            for inst in blk.instructions:
                si = inst.sync_info
                if si is not None and len(si.on_wait) > limit:
                    waits = list(si.on_wait)
                    head, keep = waits[:-limit], waits[-limit:]
                    for i in range(0, len(head), limit):
                        n += 1
                        new.append(mybir.InstNoOp(
                            name=f"waitsplit_{n}",
                            sync_info=mybir.SyncInfo(
                                on_wait=head[i:i + limit], on_update=[]),
                            bass_nofuse=True,
                            engine=inst.engine,
                        ))
                    inst.sync_info = mybir.SyncInfo(
                        on_wait=keep, on_update=list(si.on_update))
                new.append(inst)
            blk.instructions = new


def _build_program():
    import concourse.bass as bass
    import concourse.tile as tile
    from concourse import mybir

    f32 = mybir.dt.float32
    bf16 = mybir.dt.bfloat16
    i16 = mybir.dt.int16

    nc = bass.Bass("TRN2", target_bir_lowering=False, debug=False,
                   enable_asserts=False, dynamic_dma_scratch_size=65536)

    tab_d = nc.dram_tensor("tab", [128, RANKS, 4 * C], bf16, kind="ExternalInput")
    idx_d = nc.dram_tensor("idx", [16, L2F], i16, kind="ExternalInput")
    bw_d = nc.dram_tensor("bw", [128, 4, L1F], bf16, kind="ExternalInput")
    wt_d = nc.dram_tensor("wt", [128, 2, K * O], bf16, kind="ExternalInput")
    ones4_d = nc.dram_tensor("ones4", [4, 128], bf16, kind="ExternalInput")
    bias_d = nc.dram_tensor("bias", [O, 1], f32, kind="ExternalInput")
    out_d = nc.dram_tensor("out", [O, NPIX], f32, kind="ExternalOutput")

    from concourse import library_config

    with tile.TileContext(nc) as tc:
        nc.gpsimd.load_library(library_config.mlp)   # provides DMAGatherAnt
        with tc.tile_pool(name="const", bufs=1) as cp:
            # table load gates the first gather: issue it first, split across
            # both HWDGE engines; small tensors go last so they don't steal
            # queue slots from the table.
            tab = cp.tile([128, RANKS, 4 * C], bf16, tag="tab")
            half = RANKS // 2
            nc.sync.dma_start(tab[:, :half, :], tab_d.ap()[:, :half, :])
            nc.scalar.dma_start(tab[:, half:, :], tab_d.ap()[:, half:, :])

            idxr = cp.tile([128, L2F], i16, tag="idxr")
            for grp in range(8):
                nc.sync.dma_start(idxr[16 * grp:16 * (grp + 1), :], idx_d.ap())

            w_sb = cp.tile([128, 2, K * O], bf16, tag="wsb")
            nc.scalar.dma_start(w_sb[:], wt_d.ap())

            bias_sb = cp.tile([O, 1], f32, tag="bias")
            nc.scalar.dma_start(bias_sb[:], bias_d.ap())

            ones4 = cp.tile([4, 128], bf16, tag="ones4")
            nc.scalar.dma_start(ones4[:], ones4_d.ap())

            bw = cp.tile([128, 4, L1F], bf16, tag="bw")
            nc.scalar.dma_start(bw[:], bw_d.ap())

            GK = 3                        # taps per gather call
            GCALL = GK * BLK              # 1536 idx/call: 292 ring descs, no stall
            rg = nc.gpsimd.to_reg(GCALL)
            dma_sem = nc.alloc_semaphore("gat_dma") if USE_PREP else None

            with (
                tc.tile_pool(name="g", bufs=3) as gp,
                tc.tile_pool(name="bst", bufs=2) as bstp,
                tc.tile_pool(name="bpsum", bufs=4, space="PSUM") as bpp,
                tc.tile_pool(name="opsum", bufs=2, space="PSUM") as opp,
                tc.tile_pool(name="val", bufs=4) as vp,
                tc.tile_pool(name="ob", bufs=2) as obp,
            ):
                for t in range(NBLK):
                    gs = []
                    for kg in range(K // GK):
                        g = gp.tile([128, 2, GCALL], bf16, tag="g")
                        s0 = t * (CALL // 16) + kg * (GCALL // 16)
                        nc.gpsimd.dma_gather(
                            g[:], tab[:], idxr[:, s0:s0 + GCALL // 16],
                            GCALL, rg, 4 * C,
                            transpose=True, single_packet=False,
                            sbuf_tokens_per_rank=128,
                            sbuf_free_dim_per_rank=512)
                        gs.append(g)

                    # stage this block's beta rows at partitions 0-3 for PE rhs
                    bst = bstp.tile([4, CALL], bf16, tag="bst")
                    src = bw[8 * t:8 * (t + 1), :, :]
                    for q in range(4):
                        nc.sync.dma_start(bst[q:q + 1, :], src[:, q, :])

                    ops = opp.tile([O, BLK], f32, tag="ops")
                    for k in range(K):
                        sl = slice(k * BLK, (k + 1) * BLK)
                        gsl = slice((k % GK) * BLK, (k % GK + 1) * BLK)
                        g = gs[k // GK]
                        bp = bpp.tile([128, BLK], f32, tag="bp")
                        nc.tensor.matmul(bp[:], ones4[:], bst[:, sl],
                                         start=True, stop=True)
                        vA = vp.tile([128, BLK], bf16, tag="v")
                        nc.vector.tensor_mul(vA[:], g[:, 0, gsl], bp[:])
                        vB = vp.tile([128, BLK], bf16, tag="v")
                        nc.vector.tensor_mul(vB[:], g[:, 1, gsl], bp[:])
                        nc.tensor.matmul(ops[:], w_sb[:, 0, k * O:(k + 1) * O],
                                         vA[:], start=(k == 0), stop=False)
                        nc.tensor.matmul(ops[:], w_sb[:, 1, k * O:(k + 1) * O],
                                         vB[:], start=False, stop=(k == K - 1))

                    ob = obp.tile([O, BLK], f32, tag="ob")
                    nc.scalar.add(ob[:], ops[:], bias_sb[:, 0:1])
                    nc.sync.dma_start(out_d.ap()[:, t * BLK:(t + 1) * BLK], ob[:])

    _split_excess_waits(nc)
    from concourse.library_overlay import lower_extended_insts
    lower_extended_insts(nc)
    return nc


def _host_prep(input, offset, mask, weight, bias):
    x = np.asarray(input, np.float32)
    off = np.asarray(offset, np.float32)
    msk = np.asarray(mask, np.float32)
    w = np.asarray(weight, np.float32)
    b = np.asarray(bias, np.float32)

    amax = float(np.abs(off).max())
    if amax >= PAD - 1.2:
        raise ValueError(f"offset magnitude {amax} exceeds supported window")

    f32 = np.float32

    # per-sample gather tables; row = [half0: I,D01,D10,D11 x 32ch | half1]
    tabs = []
    for bb in range(B):
        E = np.zeros((C, GEXT, GEXT), f32)
        E[:, PAD:PAD + H, PAD:PAD + W] = x[bb]
        Eb = E.astype(BF16).astype(f32)
        D01 = np.zeros((C, GEXT, GEXT), f32)
        D01[:, :, :-1] = Eb[:, :, 1:] - Eb[:, :, :-1]
        D10 = np.zeros((C, GEXT, GEXT), f32)
        D10[:, :-1, :] = Eb[:, 1:, :] - Eb[:, :-1, :]
        D11 = np.zeros((C, GEXT, GEXT), f32)
        D11[:, :-1, :-1] = (Eb[:, 1:, 1:] - Eb[:, 1:, :-1]
                            - Eb[:, :-1, 1:] + Eb[:, :-1, :-1])
        planes = np.stack([Eb, D01, D10, D11], 0)[:, :, :TG, :TG]  # [4,C,TG,TG]
        # -> [TG, TG, half, plane, 32] -> rows [NROWS, 256]
        arr = planes.reshape(4, 2, 32, TG, TG).transpose(3, 4, 1, 0, 2)
        rows = np.zeros((NROWS_PAD, 4 * C), BF16)
        rows[:NROWS] = arr.reshape(NROWS, 4 * C).astype(BF16)
        tabdram = np.ascontiguousarray(
            rows.reshape(RANKS, 128, 4 * C).transpose(1, 0, 2))   # [128,165,256]
        tabs.append(tabdram)

    # conv lhsT per rank-half: [128 part=(plane,ch%32), K, O]
    wk = w.reshape(O, C, K)                      # [O, C, K]
    wtA = np.zeros((128, K, O), f32)
    wtB = np.zeros((128, K, O), f32)
    for p in range(128):
        c = p % 32
        wtA[p] = wk[:, c, :].T                   # [K, O]
        wtB[p] = wk[:, 32 + c, :].T
    wt_both = np.stack(
        [wtA.reshape(128, K * O), wtB.reshape(128, K * O)], axis=1)  # [128,2,KO]

    ones4c = np.zeros((4, 128), f32)
    for q in range(4):
        ones4c[q, 32 * q:32 * (q + 1)] = 1.0
    bias2 = np.ascontiguousarray(b.reshape(O, 1))

    karr = np.arange(K)
    p = np.arange(NPIX)
    ylo = p // W
    xloc = p % W

    def tojd(a):  # [K, NPIX] -> flat j order (t, k, ptilde)
        return np.ascontiguousarray(
            a.reshape(K, NBLK, BLK).transpose(1, 0, 2).reshape(-1))

    in_maps = []
    for core in range(8):
        bb, h = divmod(core, 2)
        yg = h * 64 + ylo                                   # [NPIX] global y
        offv = off[bb].reshape(K, 2, H, W)
        oy_kp = offv[:, 0][:, yg, xloc]                     # [K, NPIX]
        ox_kp = offv[:, 1][:, yg, xloc]
        m_kp = msk[bb][:, yg, xloc]
        by = yg[None, :] - 1 + (karr // 3)[:, None]
        bx = xloc[None, :] - 1 + (karr % 3)[:, None]

        y0 = np.floor(oy_kp)
        x0 = np.floor(ox_kp)
        ly = oy_kp - y0
        lx = ox_kp - x0
        idx_kp = (y0 + by + PAD) * TG + (x0 + bx + PAD)     # [K, NPIX]

        idxj = tojd(idx_kp).astype(np.int64)
        assert idxj.min() >= 0 and idxj.max() < NROWS
        idx16 = np.ascontiguousarray(
            idxj.reshape(NSAMP // 16, 16).T).astype(np.int16)

        mj = tojd(m_kp).astype(f32)
        lxj = tojd(lx).astype(f32)
        lyj = tojd(ly).astype(f32)
        # bw[q, plane, e]: j = (q//8)*4608 + (q%8)*576 + e
        bws = np.stack([mj, mj * lxj, mj * lyj, mj * lxj * lyj], 0)  # [4, NSAMP]
        bw_host = np.ascontiguousarray(
            bws.reshape(4, 128, L1F).transpose(1, 0, 2)).astype(BF16)

        in_maps.append({
            "tab": tabs[bb],
            "idx": idx16,
            "bw": bw_host,
            "wt": wt_both.astype(BF16),
            "ones4": ones4c.astype(BF16),
            "bias": bias2,
        })
    return in_maps


def _install_ntff_shim():
    """Provide antenv.axon_hooks (missing in this image) so trace=True works."""
    import types
    if "antenv.axon_hooks" in sys.modules:
        return
    sys.path.insert(0, "/root/.axon_site")
    from trn_agent_boot.trn_boot import _ntff_profile_via_ctypes
    hook = _ntff_profile_via_ctypes("/opt/axon/libaxon_pjrt.so")
    mod = types.ModuleType("antenv.axon_hooks")
    mod.get_axon_ntff_profile_hook = lambda: hook
    mod.set_axon_ntff_profile_hook = lambda h: None
    sys.modules["antenv.axon_hooks"] = mod


def kernel(input, offset, mask, weight, bias, _trace=False):
    if _trace:
        _install_ntff_shim()
    from concourse.bass_utils import run_bass_kernel_spmd

    if "nc" not in _CACHE:
        _CACHE["nc"] = _build_program()
    nc = _CACHE["nc"]

    in_maps = _host_prep(input, offset, mask, weight, bias)
    res = run_bass_kernel_spmd(
        nc, in_maps, core_ids=list(range(8)),
        trace=_trace,
        trace_cores=list(range(8)) if _trace else None,
    )
    kernel.last_results = res

    out = np.empty((B, O, H, W), np.float32)
    for core in range(8):
        bb, h = divmod(core, 2)
        blockout = res.results[core]["out"]       # [64, 8192] f32
        out[bb, :, h * 64:(h + 1) * 64, :] = blockout.reshape(O, 64, W)
    return out


# revision 7
# speedup vs baseline: 1.3621x; 1.0403x over previous
"""Trainium2 Bass kernel for modulated deformable conv2d (torchvision semantics).

Problem (hardcoded): input [4,64,128,128] f32, offset [4,18,128,128], mask
[4,9,128,128], weight [64,64,3,3], bias [64]; stride 1, pad 1, dil 1.

Strategy (8 NeuronCores, SPMD, no collectives):
  - Shard: core = (sample b, row-half h).  Each core computes output rows
    [h*64, h*64+64) of sample b => out slice [64, 8192] f32.
  - Bilinear gather via difference planes:
        val = I[y0,x0] + lx*D01[y0,x0] + ly*D10[y0,x0] + lx*ly*D11[y0,x0]
    Each sample point is a SINGLE 512-byte-row gather from an SBUF-resident
    table, fetched with GPSIMD dma_gather (transpose mode) so data lands
    channels-on-partitions.  Table row layout (256 bf16):
        [I(c0:32) D01(c0:32) D10(c0:32) D11(c0:32) | same for c32:64]
    so transposed rank r, partition p = (chan-half r, plane p//32, ch p%32).
  - Gather indices and beta planes (m, m*lx, m*ly, m*lx*ly) are computed on
    host and shipped as inputs; device replicates the 16-wrap indices.
  - Per 512-pixel block x 9 taps: one contraction-4 PE matmul broadcasts all
    4 betas to their 32-partition groups (PSUM), DVE multiplies the gathered
    rows by betas (reading PSUM directly), and the conv contraction
    (plane-sum + channels -> O) runs as an accumulated PE matmul chain.
  - Gathers use prepare_only + trigger so GPSIMD desc-gen for block t+1
    overlaps DMA execution of block t and compute of block t-1.
"""

import sys

if "/opt/trn_rl_repo" not in sys.path:
    sys.path.insert(0, "/opt/trn_rl_repo")

import numpy as np
import ml_dtypes

BF16 = ml_dtypes.bfloat16

# problem dims
B, C, H, W = 4, 64, 128, 128
O, K = 64, 9
PAD = 8                     # gather window margin (|offset| <= ~6.8 required)
TG = H + 2 * PAD + 1        # 145: table grid covers y,x in [-PAD, H+PAD]
GEXT = TG + 1               # 146: extended image grid (D planes read +1)
NROWS = TG * TG             # 21025
RANKS = (NROWS + 127) // 128  # 165
NROWS_PAD = RANKS * 128     # 21120
NPIX = H * W // 2           # 8192 output pixels per core
NBLK = 16                   # pixel blocks per core
BLK = NPIX // NBLK          # 512 pixels per block
CALL = K * BLK              # 4608 gather indices per block (all 9 taps)
NSAMP = K * NPIX            # 73728 sample points per core
MAGIC = 12582912.0          # kept for import compat

L1F = NSAMP // 128          # 576
L2F = NSAMP // 16           # 4608

_CACHE = {}

USE_PREP = False            # prep+trigger measured as pure overhead (+1.7us/call)


def _split_excess_waits(nc, limit=1):
    """Walrus in this image caps sync-wait commands per instruction; hoist
    excess waits onto preceding same-engine NoOps (engine streams are
    in-order, so blocking earlier on a prefix of the waits is equivalent)."""
    from concourse import mybir

    n = 0
    for fn in nc.m.functions:
        for blk in fn.blocks:
            new = []
            for inst in blk.instructions:
                si = inst.sync_info
                if si is not None and len(si.on_wait) > limit:
                    waits = list(si.on_wait)
                    head, keep = waits[:-limit], waits[-limit:]
                    for i in range(0, len(head), limit):
                        n += 1
                        new.append(mybir.InstNoOp(
                            name=f"waitsplit_{n}",
                            sync_info=mybir.SyncInfo(
                                on_wait=head[i:i + limit], on_update=[]),
                            bass_nofuse=True,
                            engine=inst.engine,
                        ))
                    inst.sync_info = mybir.SyncInfo(
                        on_wait=keep, on_update=list(si.on_update))
                new.append(inst)
            blk.instructions = new


def _build_program():
    import concourse.bass as bass
    import concourse.tile as tile
    from concourse import mybir

    f32 = mybir.dt.float32
    bf16 = mybir.dt.bfloat16
    i16 = mybir.dt.int16

    nc = bass.Bass("TRN2", target_bir_lowering=False, debug=False,
                   enable_asserts=False, dynamic_dma_scratch_size=65536)

    tab_d = nc.dram_tensor("tab", [NROWS_PAD, 4 * C], bf16, kind="ExternalInput")
    idx_d = nc.dram_tensor("idx", [16, L2F], i16, kind="ExternalInput")
    bw_d = nc.dram_tensor("bw", [128, 4, L1F], bf16, kind="ExternalInput")
    wt_d = nc.dram_tensor("wt", [128, 2, K * O], bf16, kind="ExternalInput")
    ones4_d = nc.dram_tensor("ones4", [4, 128], bf16, kind="ExternalInput")
    bias_d = nc.dram_tensor("bias", [O, 1], f32, kind="ExternalInput")
    out_d = nc.dram_tensor("out", [O, NPIX], f32, kind="ExternalOutput")

    from concourse import library_config

    with tile.TileContext(nc) as tc:
        nc.gpsimd.load_library(library_config.mlp)   # provides DMAGatherAnt
        with tc.tile_pool(name="const", bufs=1) as cp:
            # gathers read the table directly from DRAM (same GPSIMD cost
            # as SBUF source, measured) -> no table load gates the pipeline.
            idxr = cp.tile([128, L2F], i16, tag="idxr")
            for grp in range(8):
                nc.sync.dma_start(idxr[16 * grp:16 * (grp + 1), :], idx_d.ap())

            w_sb = cp.tile([128, 2, K * O], bf16, tag="wsb")
            nc.scalar.dma_start(w_sb[:], wt_d.ap())

            bias_sb = cp.tile([O, 1], f32, tag="bias")
            nc.scalar.dma_start(bias_sb[:], bias_d.ap())

            ones4 = cp.tile([4, 128], bf16, tag="ones4")
            nc.scalar.dma_start(ones4[:], ones4_d.ap())

            bw = cp.tile([128, 4, L1F], bf16, tag="bw")
            nc.scalar.dma_start(bw[:], bw_d.ap())

            GK = 6                        # taps per gather call
            GCALL = GK * BLK              # 3072 idx/call: ~580 ring descs
            NCH = NSAMP // GCALL          # 24 chunks
            rg = nc.gpsimd.to_reg(GCALL)
            dma_sem = nc.alloc_semaphore("gat_dma") if USE_PREP else None

            with (
                tc.tile_pool(name="g", bufs=3) as gp,
                tc.tile_pool(name="bst", bufs=2) as bstp,
                tc.tile_pool(name="bpsum", bufs=4, space="PSUM") as bpp,
                tc.tile_pool(name="opsum", bufs=2, space="PSUM") as opp,
                tc.tile_pool(name="val", bufs=4) as vp,
                tc.tile_pool(name="ob", bufs=2) as obp,
            ):
                gtiles = {}

                def get_chunk(c):
                    if c not in gtiles:
                        g = gp.tile([128, 2, GCALL], bf16, tag="g")
                        s0 = c * (GCALL // 16)
                        nc.gpsimd.dma_gather(
                            g[:], tab_d.ap(), idxr[:, s0:s0 + GCALL // 16],
                            GCALL, rg, 4 * C,
                            transpose=True, single_packet=False)
                        gtiles[c] = g
                    return gtiles[c]

                for t in range(NBLK):
                    # stage this block's beta rows at partitions 0-3 for PE rhs
                    bst = bstp.tile([4, CALL], bf16, tag="bst")
                    src = bw[8 * t:8 * (t + 1), :, :]
                    for q in range(4):
                        nc.sync.dma_start(bst[q:q + 1, :], src[:, q, :])

                    ops = opp.tile([O, BLK], f32, tag="ops")
                    for k in range(K):
                        sl = slice(k * BLK, (k + 1) * BLK)
                        tk = t * K + k
                        g = get_chunk(tk // GK)
                        gsl = slice((tk % GK) * BLK, (tk % GK + 1) * BLK)
                        bp = bpp.tile([128, BLK], f32, tag="bp")
                        nc.tensor.matmul(bp[:], ones4[:], bst[:, sl],
                                         start=True, stop=True)
                        vA = vp.tile([128, BLK], bf16, tag="v")
                        nc.vector.tensor_mul(vA[:], g[:, 0, gsl], bp[:])
                        vB = vp.tile([128, BLK], bf16, tag="v")
                        nc.vector.tensor_mul(vB[:], g[:, 1, gsl], bp[:])
                        nc.tensor.matmul(ops[:], w_sb[:, 0, k * O:(k + 1) * O],
                                         vA[:], start=(k == 0), stop=False)
                        nc.tensor.matmul(ops[:], w_sb[:, 1, k * O:(k + 1) * O],
                                         vB[:], start=False, stop=(k == K - 1))

                    ob = obp.tile([O, BLK], f32, tag="ob")
                    nc.scalar.add(ob[:], ops[:], bias_sb[:, 0:1])
                    nc.sync.dma_start(out_d.ap()[:, t * BLK:(t + 1) * BLK], ob[:])

    _split_excess_waits(nc)
    from concourse.library_overlay import lower_extended_insts
    lower_extended_insts(nc)
    return nc


def _host_prep(input, offset, mask, weight, bias):
    x = np.asarray(input, np.float32)
    off = np.asarray(offset, np.float32)
    msk = np.asarray(mask, np.float32)
    w = np.asarray(weight, np.float32)
    b = np.asarray(bias, np.float32)

    amax = float(np.abs(off).max())
    if amax >= PAD - 1.2:
        raise ValueError(f"offset magnitude {amax} exceeds supported window")

    f32 = np.float32

    # per-sample gather tables; row = [half0: I,D01,D10,D11 x 32ch | half1]
    tabs = []
    for bb in range(B):
        E = np.zeros((C, GEXT, GEXT), f32)
        E[:, PAD:PAD + H, PAD:PAD + W] = x[bb]
        Eb = E.astype(BF16).astype(f32)
        D01 = np.zeros((C, GEXT, GEXT), f32)
        D01[:, :, :-1] = Eb[:, :, 1:] - Eb[:, :, :-1]
        D10 = np.zeros((C, GEXT, GEXT), f32)
        D10[:, :-1, :] = Eb[:, 1:, :] - Eb[:, :-1, :]
        D11 = np.zeros((C, GEXT, GEXT), f32)
        D11[:, :-1, :-1] = (Eb[:, 1:, 1:] - Eb[:, 1:, :-1]
                            - Eb[:, :-1, 1:] + Eb[:, :-1, :-1])
        planes = np.stack([Eb, D01, D10, D11], 0)[:, :, :TG, :TG]  # [4,C,TG,TG]
        # -> [TG, TG, half, plane, 32] -> rows [NROWS, 256]
        arr = planes.reshape(4, 2, 32, TG, TG).transpose(3, 4, 1, 0, 2)
        rows = np.zeros((NROWS_PAD, 4 * C), BF16)
        rows[:NROWS] = arr.reshape(NROWS, 4 * C).astype(BF16)
        tabs.append(rows)

    # conv lhsT per rank-half: [128 part=(plane,ch%32), K, O]
    wk = w.reshape(O, C, K)                      # [O, C, K]
    wtA = np.zeros((128, K, O), f32)
    wtB = np.zeros((128, K, O), f32)
    for p in range(128):
        c = p % 32
        wtA[p] = wk[:, c, :].T                   # [K, O]
        wtB[p] = wk[:, 32 + c, :].T
    wt_both = np.stack(
        [wtA.reshape(128, K * O), wtB.reshape(128, K * O)], axis=1)  # [128,2,KO]

    ones4c = np.zeros((4, 128), f32)
    for q in range(4):
        ones4c[q, 32 * q:32 * (q + 1)] = 1.0
    bias2 = np.ascontiguousarray(b.reshape(O, 1))

    karr = np.arange(K)
    p = np.arange(NPIX)
    ylo = p // W
    xloc = p % W

    def tojd(a):  # [K, NPIX] -> flat j order (t, k, ptilde)
        return np.ascontiguousarray(
            a.reshape(K, NBLK, BLK).transpose(1, 0, 2).reshape(-1))

    in_maps = []
    for core in range(8):
        bb, h = divmod(core, 2)
        yg = h * 64 + ylo                                   # [NPIX] global y
        offv = off[bb].reshape(K, 2, H, W)
        oy_kp = offv[:, 0][:, yg, xloc]                     # [K, NPIX]
        ox_kp = offv[:, 1][:, yg, xloc]
        m_kp = msk[bb][:, yg, xloc]
        by = yg[None, :] - 1 + (karr // 3)[:, None]
        bx = xloc[None, :] - 1 + (karr % 3)[:, None]

        y0 = np.floor(oy_kp)
        x0 = np.floor(ox_kp)
        ly = oy_kp - y0
        lx = ox_kp - x0
        idx_kp = (y0 + by + PAD) * TG + (x0 + bx + PAD)     # [K, NPIX]

        idxj = tojd(idx_kp).astype(np.int64)
        assert idxj.min() >= 0 and idxj.max() < NROWS
        idx16 = np.ascontiguousarray(
            idxj.reshape(NSAMP // 16, 16).T).astype(np.int16)

        mj = tojd(m_kp).astype(f32)
        lxj = tojd(lx).astype(f32)
        lyj = tojd(ly).astype(f32)
        # bw[q, plane, e]: j = (q//8)*4608 + (q%8)*576 + e
        bws = np.stack([mj, mj * lxj, mj * lyj, mj * lxj * lyj], 0)  # [4, NSAMP]
        bw_host = np.ascontiguousarray(
            bws.reshape(4, 128, L1F).transpose(1, 0, 2)).astype(BF16)

        in_maps.append({
            "tab": tabs[bb],
            "idx": idx16,
            "bw": bw_host,
            "wt": wt_both.astype(BF16),
            "ones4": ones4c.astype(BF16),
            "bias": bias2,
        })
    return in_maps


def _install_ntff_shim():
    """Provide antenv.axon_hooks (missing in this image) so trace=True works."""
    import types
    if "antenv.axon_hooks" in sys.modules:
        return
    sys.path.insert(0, "/root/.axon_site")
    from trn_agent_boot.trn_boot import _ntff_profile_via_ctypes
    hook = _ntff_profile_via_ctypes("/opt/axon/libaxon_pjrt.so")
    mod = types.ModuleType("antenv.axon_hooks")
    mod.get_axon_ntff_profile_hook = lambda: hook
    mod.set_axon_ntff_profile_hook = lambda h: None
    sys.modules["antenv.axon_hooks"] = mod


def kernel(input, offset, mask, weight, bias, _trace=False):
    if _trace:
        _install_ntff_shim()
    from concourse.bass_utils import run_bass_kernel_spmd

    if "nc" not in _CACHE:
        _CACHE["nc"] = _build_program()
    nc = _CACHE["nc"]

    in_maps = _host_prep(input, offset, mask, weight, bias)
    res = run_bass_kernel_spmd(
        nc, in_maps, core_ids=list(range(8)),
        trace=_trace,
        trace_cores=list(range(8)) if _trace else None,
    )
    kernel.last_results = res

    out = np.empty((B, O, H, W), np.float32)
    for core in range(8):
        bb, h = divmod(core, 2)
        blockout = res.results[core]["out"]       # [64, 8192] f32
        out[bb, :, h * 64:(h + 1) * 64, :] = blockout.reshape(O, 64, W)
    return out
